# revision 3
# baseline (speedup 1.0000x reference)
"""Multi-head self-attention (B=4, T=2048, D=1024, H=16) on 8 trn2 cores.

Sharding: core = b * 2 + g  (b = batch 0..3, g = head-group 0..1, 8 heads each).
Each core computes, for its (batch, 8-head group):
  Q^T,K^T [hd=512, T] and V [T, hd] from x^T (host-pretransposed) via f32r matmuls,
  per head: E = exp((K^T_h)^T @ Q^T_h / 8)  in [s, t] layout,
  ctx^T/denominator via PV matmul with a ones-row appended to V,
  out-projection partial y_part = ctx^T.T @ w_out[g rows] + b_out (g==0 only).
Host sums the two head-group partials per batch.

All matmuls run in float32r (TF32-like, 1 cycle/row at N>=512).
"""

import numpy as np
import concourse.bass as bass
import concourse.bacc as bacc
import concourse.mybir as mybir
import concourse.tile as tile
from concourse.bass_utils import run_bass_kernel_spmd

B, T, D = 4, 2048, 1024
H, DK = 16, 64
G = 2                 # head groups
HPG = H // G          # heads per core = 8
HD = HPG * DK         # 512
NCORES = B * G        # 8
SCALE = 1.0 / float(np.sqrt(DK))

F32 = mybir.dt.float32
F32R = mybir.dt.float32r
Ident = mybir.ActivationFunctionType.Identity
Exp = mybir.ActivationFunctionType.Exp

NCC = D // 128        # 8 contraction chunks
NDT = HD // 128       # 4 d-tiles of Q/K per core
NTT = T // 128        # 16 t-tiles
NTB = T // 512        # 4 t-blocks of 512
NSI = T // 128        # 16 s-tiles
VW = HPG * (DK + 1)   # 520: V padded with a ones column per head


def build_program():
    nc = bacc.Bacc("TRN2", target_bir_lowering=False, debug=False)

    xt = nc.dram_tensor("xt", [D, T], F32, kind="ExternalInput").ap()
    wq = nc.dram_tensor("wq", [D, HD], F32, kind="ExternalInput").ap()
    wk = nc.dram_tensor("wk", [D, HD], F32, kind="ExternalInput").ap()
    wv = nc.dram_tensor("wv", [D, HD], F32, kind="ExternalInput").ap()
    bq = nc.dram_tensor("bq", [HD, 1], F32, kind="ExternalInput").ap()
    bk = nc.dram_tensor("bk", [HD, 1], F32, kind="ExternalInput").ap()
    bv = nc.dram_tensor("bv", [128, HD], F32, kind="ExternalInput").ap()
    wo = nc.dram_tensor("wo", [HD, D], F32, kind="ExternalInput").ap()
    bo = nc.dram_tensor("bo", [128, D], F32, kind="ExternalInput").ap()
    y = nc.dram_tensor("y", [T, D], F32, kind="ExternalOutput").ap()

    with tile.TileContext(nc) as tc:
        with tc.tile_pool(name="persist", bufs=1) as pp:
            # persistent QKV outputs
            qT = [pp.tile([128, T], F32R, name=f"qT{i}", tag=f"qT{i}") for i in range(NDT)]
            kT = [pp.tile([128, T], F32R, name=f"kT{i}", tag=f"kT{i}") for i in range(NDT)]
            vv = [pp.tile([128, VW], F32R, name=f"v{i}", tag=f"v{i}") for i in range(NTT)]
            ones_f32 = pp.tile([128, 64], F32, name="ones_f32", tag="ones_f32")
            nc.vector.memset(ones_f32[:], 1.0)
            ones64 = pp.tile([1, 64], F32R, name="ones64", tag="ones64")
            nc.vector.tensor_copy(ones64[:], ones_f32[0:1, :])
            bq_sb = [pp.tile([128, 1], F32, name=f"bq{i}", tag=f"bq{i}") for i in range(NDT)]
            bk_sb = [pp.tile([128, 1], F32, name=f"bk{i}", tag=f"bk{i}") for i in range(NDT)]
            bv_sb = pp.tile([128, HD], F32, name="bv_sb", tag="bv_sb")
            for i in range(NDT):
                nc.sync.dma_start(bq_sb[i][:], bq[i * 128:(i + 1) * 128, :])
                nc.sync.dma_start(bk_sb[i][:], bk[i * 128:(i + 1) * 128, :])
            nc.sync.dma_start(bv_sb[:], bv[:])

            # ---------------- Phase 1: QKV projections ----------------
            with tc.tile_pool(name="p1", bufs=1) as p1, \
                 tc.tile_pool(name="p1ps", bufs=1, space="PSUM") as p1ps:
                wq_sb = [p1.tile([128, HD], F32R, name=f"wq{c}", tag=f"wq{c}") for c in range(NCC)]
                wk_sb = [p1.tile([128, HD], F32R, name=f"wk{c}", tag=f"wk{c}") for c in range(NCC)]
                wv_sb = [p1.tile([128, HD], F32R, name=f"wv{c}", tag=f"wv{c}") for c in range(NCC)]
                for c in range(NCC):
                    cs = slice(c * 128, (c + 1) * 128)
                    nc.sync.dma_start(wq_sb[c][:], wq[cs, :].bitcast(F32R))
                    nc.sync.dma_start(wk_sb[c][:], wk[cs, :].bitcast(F32R))
                    nc.sync.dma_start(wv_sb[c][:], wv[cs, :].bitcast(F32R))

                for half in range(2):
                    hslice = slice(half * 1024, (half + 1) * 1024)
                    xt_half = []
                    for c in range(NCC):
                        xh = p1.tile([128, 1024], F32R, name=f"xt_h{half}_{c}",
                                     tag="xt", bufs=9)
                        nc.sync.dma_start(xh[:], xt[c * 128:(c + 1) * 128, hslice].bitcast(F32R))
                        xt_half.append(xh)

                    # Q^T and K^T d-tiles:  psum[d,t] += w[c,d]^T.T @ xt[c,t]
                    for w_sb, b_sb, out_t in ((wq_sb, bq_sb, qT), (wk_sb, bk_sb, kT)):
                        for dt_i in range(NDT):
                            for tci in range(2):
                                ps = p1ps.tile([128, 512], F32, name="qk_ps",
                                               tag="qk_ps", bufs=3)
                                for c in range(NCC):
                                    nc.tensor.matmul(
                                        ps[:],
                                        w_sb[c][:, dt_i * 128:(dt_i + 1) * 128],
                                        xt_half[c][:, tci * 512:(tci + 1) * 512],
                                        start=(c == 0), stop=(c == NCC - 1))
                                nc.scalar.activation(
                                    out_t[dt_i][:, half * 1024 + tci * 512:
                                                half * 1024 + (tci + 1) * 512],
                                    ps[:], Ident, bias=b_sb[dt_i][:])

                    # V t-tiles: psum[t,d] += xt[c,t-tile].T @ wv[c,:]
                    for tt in range(8):
                        ps = p1ps.tile([128, 512], F32, name="v_ps", tag="qk_ps", bufs=3)
                        for c in range(NCC):
                            nc.tensor.matmul(
                                ps[:],
                                xt_half[c][:, tt * 128:(tt + 1) * 128],
                                wv_sb[c][:],
                                start=(c == 0), stop=(c == NCC - 1))
                        vt = vv[half * 8 + tt]
                        v3 = vt[:].rearrange("p (h e) -> p h e", e=DK + 1)
                        with nc.allow_low_precision(reason="f32r rounding for PE"):
                            nc.vector.tensor_add(
                                v3[:, :, 0:DK],
                                ps[:].rearrange("p (h e) -> p h e", e=DK),
                                bv_sb[:].rearrange("p (h e) -> p h e", e=DK))
                        nc.vector.tensor_copy(
                            v3[:, :, DK:DK + 1],
                            ones_f32[:, 0:HPG].rearrange("p (h e) -> p h e", e=1))

            # ---------------- Phase 2: attention ----------------
            with tc.tile_pool(name="p2ctx", bufs=1) as p2c:
                ctx = [p2c.tile([128, T], F32R, name=f"ctx{i}", tag=f"ctx{i}")
                       for i in range(NDT)]
                with tc.tile_pool(name="p2", bufs=1) as p2, \
                     tc.tile_pool(name="p2ps", bufs=1, space="PSUM") as p2ps:
                    for h in range(HPG):
                        ti = h // 2
                        ro = (h % 2) * 64
                        for tb in range(NTB):
                            tbs = slice(tb * 512, (tb + 1) * 512)
                            pc = p2ps.tile([65, 512], F32, name="ctx_ps",
                                           tag="ctx_ps", bufs=2)
                            for si in range(NSI):
                                ps = p2ps.tile([128, 512], F32, name="sc_ps",
                                               tag="sc_ps", bufs=2)
                                nc.tensor.matmul(
                                    ps[:],
                                    kT[ti][ro:ro + 64, si * 128:(si + 1) * 128],
                                    qT[ti][ro:ro + 64, tbs],
                                    start=True, stop=True)
                                et = p2.tile([128, 512], F32R, name="e_t",
                                             tag="e_t", bufs=6)
                                nc.scalar.activation(et[:], ps[:], Exp, scale=SCALE)
                                nc.tensor.matmul(
                                    pc[:],
                                    vv[si][:, h * (DK + 1):(h + 1) * (DK + 1)],
                                    et[:],
                                    start=(si == 0), stop=(si == NSI - 1))
                            # normalize: ctx[d,t] = pc[d,t] / pc[64,t]
                            r = p2.tile([1, 512], F32R, name="r_t", tag="r_t", bufs=2)
                            with nc.allow_low_precision(reason="f32r rounding for PE"):
                                nc.vector.reciprocal(r[:], pc[64:65, :])
                            pb = p2ps.tile([64, 512], F32, name="rb_ps",
                                           tag="rb_ps", bufs=2)
                            nc.tensor.matmul(pb[:], ones64[:], r[:],
                                             start=True, stop=True)
                            cu = p2.tile([64, 512], F32, name="cu_t", tag="cu_t", bufs=2)
                            nc.scalar.activation(cu[:], pc[0:64, :], Ident)
                            with nc.allow_low_precision(reason="f32r rounding for PE"):
                                nc.vector.tensor_mul(
                                    ctx[ti][ro:ro + 64, tbs], cu[:], pb[:])

                # ---------------- Phase 3: out-projection ----------------
                with tc.tile_pool(name="p3", bufs=1) as p3, \
                     tc.tile_pool(name="p3ps", bufs=1, space="PSUM") as p3ps:
                    wo_sb = [p3.tile([128, D], F32R, name=f"wo{c}", tag=f"wo{c}")
                             for c in range(NDT)]
                    for c in range(NDT):
                        nc.sync.dma_start(wo_sb[c][:],
                                          wo[c * 128:(c + 1) * 128, :].bitcast(F32R))
                    bo_sb = p3.tile([128, D], F32, name="bo_sb", tag="bo_sb")
                    nc.sync.dma_start(bo_sb[:], bo[:])
                    for tt in range(NTT):
                        py = p3ps.tile([128, D], F32, name="y_ps", tag="y_ps", bufs=2)
                        for ci in range(NDT):
                            for nh in range(2):
                                nc.tensor.matmul(
                                    py[:, nh * 512:(nh + 1) * 512],
                                    ctx[ci][:, tt * 128:(tt + 1) * 128],
                                    wo_sb[ci][:, nh * 512:(nh + 1) * 512],
                                    start=(ci == 0), stop=(ci == NDT - 1))
                        yt = p3.tile([128, D], F32, name="y_t", tag="y_t", bufs=3)
                        nc.vector.tensor_add(yt[:], py[:], bo_sb[:])
                        nc.sync.dma_start(y[tt * 128:(tt + 1) * 128, :], yt[:])

    nc.compile()
    return nc


_PROGRAM = None


def _get_program():
    global _PROGRAM
    if _PROGRAM is None:
        _PROGRAM = build_program()
    return _PROGRAM


def make_in_maps(x, w_qkv, b_qkv, w_out, b_out):
    x = np.ascontiguousarray(np.asarray(x, dtype=np.float32))
    w_qkv = np.asarray(w_qkv, dtype=np.float32)
    b_qkv = np.asarray(b_qkv, dtype=np.float32)
    w_out = np.asarray(w_out, dtype=np.float32)
    b_out = np.asarray(b_out, dtype=np.float32)

    in_maps = []
    for core in range(NCORES):
        b, g = divmod(core, G)
        gs = slice(g * HD, (g + 1) * HD)
        bo_part = b_out if g == 0 else np.zeros_like(b_out)
        in_maps.append({
            "xt": np.ascontiguousarray(x[b].T),
            "wq": np.ascontiguousarray(w_qkv[:, 0 * D:1 * D][:, gs]),
            "wk": np.ascontiguousarray(w_qkv[:, 1 * D:2 * D][:, gs]),
            "wv": np.ascontiguousarray(w_qkv[:, 2 * D:3 * D][:, gs]),
            "bq": np.ascontiguousarray(b_qkv[0 * D:1 * D][gs].reshape(HD, 1)),
            "bk": np.ascontiguousarray(b_qkv[1 * D:2 * D][gs].reshape(HD, 1)),
            "bv": np.ascontiguousarray(
                np.broadcast_to(b_qkv[2 * D:3 * D][gs], (128, HD))),
            "wo": np.ascontiguousarray(w_out[gs, :]),
            "bo": np.ascontiguousarray(np.broadcast_to(bo_part, (128, D))),
        })
    return in_maps


def run(inputs, trace=False, tmpdir=None):
    nc = _get_program()
    in_maps = make_in_maps(**inputs)
    res = run_bass_kernel_spmd(nc, in_maps, list(range(NCORES)),
                               trace=trace, tmpdir=tmpdir)
    parts = [np.asarray(res.results[c]["y"]) for c in range(NCORES)]
    out = np.empty((B, T, D), dtype=np.float32)
    for b in range(B):
        out[b] = parts[b * G + 0] + parts[b * G + 1]
    return out, res


def kernel(**inputs) -> np.ndarray:
    out, _ = run(inputs, trace=False)
    return out


# revision 5
# speedup vs baseline: 1.0830x; 1.0830x over previous
"""Multi-head self-attention (B=4, T=2048, D=1024, H=16) on 8 trn2 cores.

Sharding: core = b * 2 + g  (b = batch 0..3, g = head-group 0..1, 8 heads each).
Each core computes, for its (batch, 8-head group):
  Q^T,K^T [hd=512, T] and V [T, hd] from x^T (host-pretransposed) via f32r matmuls,
  per head: E = exp((K^T_h)^T @ Q^T_h / 8)  in [s, t] layout,
  ctx^T/denominator via PV matmul with a ones-row appended to V,
  out-projection partial y_part = ctx^T.T @ w_out[g rows] + b_out (g==0 only).
Host sums the two head-group partials per batch.

All matmuls run in float32r (TF32-like, 1 cycle/row at N>=512).
"""

import numpy as np
import concourse.bass as bass
import concourse.bacc as bacc
import concourse.mybir as mybir
import concourse.tile as tile
from concourse.bass_utils import run_bass_kernel_spmd

B, T, D = 4, 2048, 1024
H, DK = 16, 64
G = 2                 # head groups
HPG = H // G          # heads per core = 8
HD = HPG * DK         # 512
NCORES = B * G        # 8
SCALE = 1.0 / float(np.sqrt(DK))

F32 = mybir.dt.float32
F32R = mybir.dt.float32r
Ident = mybir.ActivationFunctionType.Identity
Exp = mybir.ActivationFunctionType.Exp

NCC = D // 128        # 8 contraction chunks
NDT = HD // 128       # 4 d-tiles of Q/K per core
NTT = T // 128        # 16 t-tiles
NTB = T // 512        # 4 t-blocks of 512
NSI = T // 128        # 16 s-tiles
VW = HPG * (DK + 1)   # 520: V padded with a ones column per head


def build_program():
    nc = bacc.Bacc("TRN2", target_bir_lowering=False, debug=False)

    xt = nc.dram_tensor("xt", [D, T], F32, kind="ExternalInput").ap()
    wq = nc.dram_tensor("wq", [D, HD], F32, kind="ExternalInput").ap()
    wk = nc.dram_tensor("wk", [D, HD], F32, kind="ExternalInput").ap()
    wv = nc.dram_tensor("wv", [D, HD], F32, kind="ExternalInput").ap()
    bq = nc.dram_tensor("bq", [HD, 1], F32, kind="ExternalInput").ap()
    bk = nc.dram_tensor("bk", [HD, 1], F32, kind="ExternalInput").ap()
    bv = nc.dram_tensor("bv", [128, HD], F32, kind="ExternalInput").ap()
    wo = nc.dram_tensor("wo", [HD, D], F32, kind="ExternalInput").ap()
    bo = nc.dram_tensor("bo", [128, D], F32, kind="ExternalInput").ap()
    y = nc.dram_tensor("y", [T, D], F32, kind="ExternalOutput").ap()

    with tile.TileContext(nc) as tc:
        with tc.tile_pool(name="persist", bufs=1) as pp:
            # persistent QKV outputs
            qT = [pp.tile([128, T], F32R, name=f"qT{i}", tag=f"qT{i}") for i in range(NDT)]
            kT = [pp.tile([128, T], F32R, name=f"kT{i}", tag=f"kT{i}") for i in range(NDT)]
            vv = [pp.tile([128, VW], F32R, name=f"v{i}", tag=f"v{i}") for i in range(NTT)]
            # ---------------- Phase 1: QKV projections ----------------
            with tc.tile_pool(name="p1", bufs=1) as p1, \
                 tc.tile_pool(name="p1ps", bufs=1, space="PSUM") as p1ps:
                ones_f32 = p1.tile([128, 64], F32, name="ones_f32", tag="ones_f32")
                nc.vector.memset(ones_f32[:], 1.0)
                bq_sb = [p1.tile([128, 1], F32, name=f"bq{i}", tag=f"bq{i}") for i in range(NDT)]
                bk_sb = [p1.tile([128, 1], F32, name=f"bk{i}", tag=f"bk{i}") for i in range(NDT)]
                bv_sb = p1.tile([128, HD], F32, name="bv_sb", tag="bv_sb")
                for i in range(NDT):
                    nc.sync.dma_start(bq_sb[i][:], bq[i * 128:(i + 1) * 128, :])
                    nc.sync.dma_start(bk_sb[i][:], bk[i * 128:(i + 1) * 128, :])
                nc.sync.dma_start(bv_sb[:], bv[:])
                wq_sb = [p1.tile([128, HD], F32R, name=f"wq{c}", tag=f"wq{c}") for c in range(NCC)]
                wk_sb = [p1.tile([128, HD], F32R, name=f"wk{c}", tag=f"wk{c}") for c in range(NCC)]
                wv_sb = [p1.tile([128, HD], F32R, name=f"wv{c}", tag=f"wv{c}") for c in range(NCC)]
                for c in range(NCC):
                    nc.sync.dma_start(wq_sb[c][:],
                                      wq[c * 128:(c + 1) * 128, :].bitcast(F32R))
                for c in range(NCC):
                    nc.sync.dma_start(wk_sb[c][:],
                                      wk[c * 128:(c + 1) * 128, :].bitcast(F32R))
                    nc.sync.dma_start(wv_sb[c][:],
                                      wv[c * 128:(c + 1) * 128, :].bitcast(F32R))

                for half in range(2):
                    hslice = slice(half * 1024, (half + 1) * 1024)
                    xt_half = []
                    for c in range(NCC):
                        xh = p1.tile([128, 1024], F32R, name=f"xt_h{half}_{c}",
                                     tag="xt", bufs=9)
                        nc.sync.dma_start(xh[:], xt[c * 128:(c + 1) * 128, hslice].bitcast(F32R))
                        xt_half.append(xh)

                    # Q^T and K^T d-tiles:  psum[d,t] += w[c,d]^T.T @ xt[c,t]
                    for w_sb, b_sb, out_t in ((wq_sb, bq_sb, qT), (wk_sb, bk_sb, kT)):
                        for dt_i in range(NDT):
                            for tci in range(2):
                                ps = p1ps.tile([128, 512], F32, name="qk_ps",
                                               tag="qk_ps", bufs=3)
                                for c in range(NCC):
                                    nc.tensor.matmul(
                                        ps[:],
                                        w_sb[c][:, dt_i * 128:(dt_i + 1) * 128],
                                        xt_half[c][:, tci * 512:(tci + 1) * 512],
                                        start=(c == 0), stop=(c == NCC - 1))
                                nc.scalar.activation(
                                    out_t[dt_i][:, half * 1024 + tci * 512:
                                                half * 1024 + (tci + 1) * 512],
                                    ps[:], Ident, bias=b_sb[dt_i][:])

                    # V t-tiles: psum[t,d] += xt[c,t-tile].T @ wv[c,:]
                    for tt in range(8):
                        ps = p1ps.tile([128, 512], F32, name="v_ps", tag="qk_ps", bufs=3)
                        for c in range(NCC):
                            nc.tensor.matmul(
                                ps[:],
                                xt_half[c][:, tt * 128:(tt + 1) * 128],
                                wv_sb[c][:],
                                start=(c == 0), stop=(c == NCC - 1))
                        vt = vv[half * 8 + tt]
                        v3 = vt[:].rearrange("p (h e) -> p h e", e=DK + 1)
                        with nc.allow_low_precision(reason="f32r rounding for PE"):
                            nc.vector.tensor_add(
                                v3[:, :, 0:DK],
                                ps[:].rearrange("p (h e) -> p h e", e=DK),
                                bv_sb[:].rearrange("p (h e) -> p h e", e=DK))
                        nc.vector.tensor_copy(
                            v3[:, :, DK:DK + 1],
                            ones_f32[:, 0:HPG].rearrange("p (h e) -> p h e", e=1))

            # ---------------- Phase 2: attention ----------------
            with tc.tile_pool(name="p2ctx", bufs=1) as p2c:
                ctx = [p2c.tile([128, T], F32R, name=f"ctx{i}", tag=f"ctx{i}")
                       for i in range(NDT)]
                with tc.tile_pool(name="p2", bufs=1) as p2, \
                     tc.tile_pool(name="p2ps", bufs=1, space="PSUM") as p2ps:
                    # software-pipelined: block k emits scores(k) interleaved
                    # with PV(k-1), so PE never stalls on ACT exp.
                    blocks = [(h, tb) for h in range(HPG) for tb in range(NTB)]
                    prev = None  # (h, tb, e_list, pc)

                    def emit_pv(prev, si):
                        h, tb, e_list, pc = prev
                        nc.tensor.matmul(
                            pc[:],
                            vv[si][:, h * (DK + 1):(h + 1) * (DK + 1)],
                            e_list[si][:],
                            start=(si == 0), stop=(si == NSI - 1))

                    def emit_norm(prev):
                        # ctx[d,t] = pc[d,t] / pc[64,t]
                        h, tb, e_list, pc = prev
                        ti, ro = h // 2, (h % 2) * 64
                        tbs = slice(tb * 512, (tb + 1) * 512)
                        r = p2.tile([1, 512], F32, name="r_t", tag="r_t", bufs=2)
                        nc.vector.reciprocal(r[:], pc[64:65, :])
                        rb = p2.tile([64, 512], F32, name="rb_t", tag="rb_t", bufs=2)
                        nc.gpsimd.partition_broadcast(rb[:], r[:])
                        with nc.allow_low_precision(reason="f32r round for PE"):
                            nc.vector.tensor_mul(
                                ctx[ti][ro:ro + 64, tbs], pc[0:64, :], rb[:])

                    for h, tb in blocks:
                        ti, ro = h // 2, (h % 2) * 64
                        tbs = slice(tb * 512, (tb + 1) * 512)
                        e_list = []
                        pc = p2ps.tile([65, 512], F32, name="ctx_ps",
                                       tag="ctx_ps", bufs=2)
                        for si in range(NSI):
                            ps = p2ps.tile([128, 512], F32, name="sc_ps",
                                           tag="sc_ps", bufs=4)
                            nc.tensor.matmul(
                                ps[:],
                                kT[ti][ro:ro + 64, si * 128:(si + 1) * 128],
                                qT[ti][ro:ro + 64, tbs],
                                start=True, stop=True)
                            et = p2.tile([128, 512], F32R, name="e_t",
                                         tag="e_t", bufs=20)
                            nc.scalar.activation(et[:], ps[:], Exp, scale=SCALE)
                            e_list.append(et)
                            if prev is not None:
                                emit_pv(prev, si)
                        if prev is not None:
                            emit_norm(prev)
                        prev = (h, tb, e_list, pc)
                    for si in range(NSI):
                        emit_pv(prev, si)
                    emit_norm(prev)

                # ---------------- Phase 3: out-projection ----------------
                with tc.tile_pool(name="p3", bufs=1) as p3, \
                     tc.tile_pool(name="p3ps", bufs=1, space="PSUM") as p3ps:
                    wo_sb = [p3.tile([128, D], F32R, name=f"wo{c}", tag=f"wo{c}")
                             for c in range(NDT)]
                    for c in range(NDT):
                        nc.sync.dma_start(wo_sb[c][:],
                                          wo[c * 128:(c + 1) * 128, :].bitcast(F32R))
                    bo_sb = p3.tile([128, D], F32, name="bo_sb", tag="bo_sb")
                    nc.sync.dma_start(bo_sb[:], bo[:])
                    for tt in range(NTT):
                        py = p3ps.tile([128, D], F32, name="y_ps", tag="y_ps", bufs=2)
                        for ci in range(NDT):
                            for nh in range(2):
                                nc.tensor.matmul(
                                    py[:, nh * 512:(nh + 1) * 512],
                                    ctx[ci][:, tt * 128:(tt + 1) * 128],
                                    wo_sb[ci][:, nh * 512:(nh + 1) * 512],
                                    start=(ci == 0), stop=(ci == NDT - 1))
                        yt = p3.tile([128, D], F32, name="y_t", tag="y_t", bufs=3)
                        nc.vector.tensor_add(yt[:], py[:], bo_sb[:])
                        nc.sync.dma_start(y[tt * 128:(tt + 1) * 128, :], yt[:])

    nc.compile()
    return nc


_PROGRAM = None


def _get_program():
    global _PROGRAM
    if _PROGRAM is None:
        _PROGRAM = build_program()
    return _PROGRAM


def make_in_maps(x, w_qkv, b_qkv, w_out, b_out):
    x = np.ascontiguousarray(np.asarray(x, dtype=np.float32))
    w_qkv = np.asarray(w_qkv, dtype=np.float32)
    b_qkv = np.asarray(b_qkv, dtype=np.float32)
    w_out = np.asarray(w_out, dtype=np.float32)
    b_out = np.asarray(b_out, dtype=np.float32)

    in_maps = []
    for core in range(NCORES):
        b, g = divmod(core, G)
        gs = slice(g * HD, (g + 1) * HD)
        bo_part = b_out if g == 0 else np.zeros_like(b_out)
        in_maps.append({
            "xt": np.ascontiguousarray(x[b].T),
            "wq": np.ascontiguousarray(w_qkv[:, 0 * D:1 * D][:, gs]),
            "wk": np.ascontiguousarray(w_qkv[:, 1 * D:2 * D][:, gs]),
            "wv": np.ascontiguousarray(w_qkv[:, 2 * D:3 * D][:, gs]),
            "bq": np.ascontiguousarray(b_qkv[0 * D:1 * D][gs].reshape(HD, 1)),
            "bk": np.ascontiguousarray(b_qkv[1 * D:2 * D][gs].reshape(HD, 1)),
            "bv": np.ascontiguousarray(
                np.broadcast_to(b_qkv[2 * D:3 * D][gs], (128, HD))),
            "wo": np.ascontiguousarray(w_out[gs, :]),
            "bo": np.ascontiguousarray(np.broadcast_to(bo_part, (128, D))),
        })
    return in_maps


def run(inputs, trace=False, tmpdir=None):
    nc = _get_program()
    in_maps = make_in_maps(**inputs)
    res = run_bass_kernel_spmd(nc, in_maps, list(range(NCORES)),
                               trace=trace, tmpdir=tmpdir)
    parts = [np.asarray(res.results[c]["y"]) for c in range(NCORES)]
    out = np.empty((B, T, D), dtype=np.float32)
    for b in range(B):
        out[b] = parts[b * G + 0] + parts[b * G + 1]
    return out, res


def kernel(**inputs) -> np.ndarray:
    out, _ = run(inputs, trace=False)
    return out


# revision 6
# speedup vs baseline: 1.8740x; 1.7304x over previous
"""Multi-head self-attention (B=4, T=2048, D=1024, H=16) on 8 trn2 cores.

Sharding: core = b * 2 + g  (b = batch 0..3, g = head-group 0..1, 8 heads each).
Each core computes, for its (batch, 8-head group):
  Q^T,K^T [hd=512, T] and V [T, hd] from x^T (host-pretransposed) via f32r matmuls,
  per head: E = exp((K^T_h)^T @ Q^T_h / 8)  in [s, t] layout,
  ctx^T/denominator via PV matmul with a ones-row appended to V,
  out-projection partial y_part = ctx^T.T @ w_out[g rows] + b_out (g==0 only).
Host sums the two head-group partials per batch.

All matmuls run in float32r (TF32-like, 1 cycle/row at N>=512).
"""

import numpy as np
import concourse.bass as bass
import concourse.bacc as bacc
import concourse.mybir as mybir
import concourse.tile as tile
from concourse.bass_utils import run_bass_kernel_spmd

B, T, D = 4, 2048, 1024
H, DK = 16, 64
G = 2                 # head groups
HPG = H // G          # heads per core = 8
HD = HPG * DK         # 512
NCORES = B * G        # 8
SCALE = 1.0 / float(np.sqrt(DK))

F32 = mybir.dt.float32
F32R = mybir.dt.float32r
Ident = mybir.ActivationFunctionType.Identity
Exp = mybir.ActivationFunctionType.Exp

NCC = D // 128        # 8 contraction chunks
NDT = HD // 128       # 4 d-tiles of Q/K per core
NTT = T // 128        # 16 t-tiles
NTB = T // 512        # 4 t-blocks of 512
NSI = T // 128        # 16 s-tiles
VW = HPG * (DK + 1)   # 520: V padded with a ones column per head


def build_program():
    nc = bacc.Bacc("TRN2", target_bir_lowering=False, debug=False)

    xt = nc.dram_tensor("xt", [D, T], F32, kind="ExternalInput").ap()
    wq = nc.dram_tensor("wq", [D, HD], F32, kind="ExternalInput").ap()
    wk = nc.dram_tensor("wk", [D, HD], F32, kind="ExternalInput").ap()
    wv = nc.dram_tensor("wv", [D, HD], F32, kind="ExternalInput").ap()
    bq = nc.dram_tensor("bq", [HD, 1], F32, kind="ExternalInput").ap()
    bk = nc.dram_tensor("bk", [HD, 1], F32, kind="ExternalInput").ap()
    bv = nc.dram_tensor("bv", [128, HD], F32, kind="ExternalInput").ap()
    wo = nc.dram_tensor("wo", [HD, D], F32, kind="ExternalInput").ap()
    bo = nc.dram_tensor("bo", [128, D], F32, kind="ExternalInput").ap()
    y = nc.dram_tensor("y", [T, D], F32, kind="ExternalOutput").ap()

    with tile.TileContext(nc) as tc:
        with tc.tile_pool(name="persist", bufs=1) as pp:
            # persistent QKV outputs
            qT = [pp.tile([128, T], F32R, name=f"qT{i}", tag=f"qT{i}") for i in range(NDT)]
            kT = [pp.tile([128, T], F32R, name=f"kT{i}", tag=f"kT{i}") for i in range(NDT)]
            vv = [pp.tile([128, VW], F32R, name=f"v{i}", tag=f"v{i}") for i in range(NTT)]
            # ---------------- Phase 1: QKV projections ----------------
            with tc.tile_pool(name="p1", bufs=1) as p1, \
                 tc.tile_pool(name="p1ps", bufs=1, space="PSUM") as p1ps:
                ones_f32 = p1.tile([128, 64], F32, name="ones_f32", tag="ones_f32")
                nc.vector.memset(ones_f32[:], 1.0)
                bq_sb = [p1.tile([128, 1], F32, name=f"bq{i}", tag=f"bq{i}") for i in range(NDT)]
                bk_sb = [p1.tile([128, 1], F32, name=f"bk{i}", tag=f"bk{i}") for i in range(NDT)]
                bv_sb = p1.tile([128, HD], F32, name="bv_sb", tag="bv_sb")
                for i in range(NDT):
                    nc.sync.dma_start(bq_sb[i][:], bq[i * 128:(i + 1) * 128, :])
                    nc.sync.dma_start(bk_sb[i][:], bk[i * 128:(i + 1) * 128, :])
                nc.sync.dma_start(bv_sb[:], bv[:])
                wq_sb = [p1.tile([128, HD], F32R, name=f"wq{c}", tag=f"wq{c}") for c in range(NCC)]
                wk_sb = [p1.tile([128, HD], F32R, name=f"wk{c}", tag=f"wk{c}") for c in range(NCC)]
                wv_sb = [p1.tile([128, HD], F32R, name=f"wv{c}", tag=f"wv{c}") for c in range(NCC)]
                for c in range(NCC):
                    nc.sync.dma_start(wq_sb[c][:],
                                      wq[c * 128:(c + 1) * 128, :].bitcast(F32R))
                for c in range(NCC):
                    nc.sync.dma_start(wk_sb[c][:],
                                      wk[c * 128:(c + 1) * 128, :].bitcast(F32R))
                    nc.sync.dma_start(wv_sb[c][:],
                                      wv[c * 128:(c + 1) * 128, :].bitcast(F32R))

                for half in range(2):
                    hslice = slice(half * 1024, (half + 1) * 1024)
                    xt_half = []
                    for c in range(NCC):
                        xh = p1.tile([128, 1024], F32R, name=f"xt_h{half}_{c}",
                                     tag="xt", bufs=9)
                        nc.sync.dma_start(xh[:], xt[c * 128:(c + 1) * 128, hslice].bitcast(F32R))
                        xt_half.append(xh)

                    # Q^T and K^T d-tiles:  psum[d,t] += w[c,d]^T.T @ xt[c,t]
                    for w_sb, b_sb, out_t in ((wq_sb, bq_sb, qT), (wk_sb, bk_sb, kT)):
                        for dt_i in range(NDT):
                            for tci in range(2):
                                ps = p1ps.tile([128, 512], F32, name="qk_ps",
                                               tag="qk_ps", bufs=3)
                                for c in range(NCC):
                                    nc.tensor.matmul(
                                        ps[:],
                                        w_sb[c][:, dt_i * 128:(dt_i + 1) * 128],
                                        xt_half[c][:, tci * 512:(tci + 1) * 512],
                                        start=(c == 0), stop=(c == NCC - 1))
                                nc.scalar.activation(
                                    out_t[dt_i][:, half * 1024 + tci * 512:
                                                half * 1024 + (tci + 1) * 512],
                                    ps[:], Ident, bias=b_sb[dt_i][:])

                    # V t-tiles: psum[t,d] += xt[c,t-tile].T @ wv[c,:]
                    for tt in range(8):
                        ps = p1ps.tile([128, 512], F32, name="v_ps", tag="qk_ps", bufs=3)
                        for c in range(NCC):
                            nc.tensor.matmul(
                                ps[:],
                                xt_half[c][:, tt * 128:(tt + 1) * 128],
                                wv_sb[c][:],
                                start=(c == 0), stop=(c == NCC - 1))
                        vt = vv[half * 8 + tt]
                        v3 = vt[:].rearrange("p (h e) -> p h e", e=DK + 1)
                        with nc.allow_low_precision(reason="f32r rounding for PE"):
                            nc.vector.tensor_add(
                                v3[:, :, 0:DK],
                                ps[:].rearrange("p (h e) -> p h e", e=DK),
                                bv_sb[:].rearrange("p (h e) -> p h e", e=DK))
                        nc.vector.tensor_copy(
                            v3[:, :, DK:DK + 1],
                            ones_f32[:, 0:HPG].rearrange("p (h e) -> p h e", e=1))

            # ---------------- Phase 2: attention ----------------
            with tc.tile_pool(name="p2ctx", bufs=1) as p2c:
                ctx = [p2c.tile([128, T], F32R, name=f"ctx{i}", tag=f"ctx{i}")
                       for i in range(NDT)]
                with tc.tile_pool(name="p2", bufs=1) as p2, \
                     tc.tile_pool(name="p2ps", bufs=1, space="PSUM") as p2ps:
                    # Software-pipelined: block k emits scores(k) interleaved
                    # with PV(k-1), so PE never stalls on ACT exp.
                    #
                    # Scores use a zero-padded same-head block-diagonal
                    # stationary [128,128] = diag(kT_h[:, sA:sA+64],
                    # kT_h[:, sA+64:sA+128]) against a doubled-Q moving tile,
                    # so every matmul streams 128 rows — K=64 matmuls keep the
                    # PE clock gate (HAM) throttled at 1.2 GHz forever.
                    zeros_t = p2.tile([128, 64], F32, name="zeros_t", tag="zeros_t")
                    nc.vector.memset(zeros_t[:], 0.0)
                    blocks = [(h, tb) for h in range(HPG) for tb in range(NTB)]
                    prev = None  # (h, tb, e_list, pc)

                    def emit_pv(prev, si):
                        h, tb, e_list, pc = prev
                        nc.tensor.matmul(
                            pc[:],
                            vv[si][:, h * (DK + 1):(h + 1) * (DK + 1)],
                            e_list[si][:],
                            start=(si == 0), stop=(si == NSI - 1))

                    def emit_norm(prev):
                        # ctx[d,t] = pc[d,t] / pc[64,t]
                        h, tb, e_list, pc = prev
                        ti, ro = h // 2, (h % 2) * 64
                        tbs = slice(tb * 512, (tb + 1) * 512)
                        r = p2.tile([1, 512], F32, name="r_t", tag="r_t", bufs=2)
                        nc.vector.reciprocal(r[:], pc[64:65, :])
                        rb = p2.tile([64, 512], F32, name="rb_t", tag="rb_t", bufs=2)
                        nc.gpsimd.partition_broadcast(rb[:], r[:])
                        with nc.allow_low_precision(reason="f32r round for PE"):
                            nc.vector.tensor_mul(
                                ctx[ti][ro:ro + 64, tbs], pc[0:64, :], rb[:])

                    kbd_tiles = []
                    for h, tb in blocks:
                        ti, ro = h // 2, (h % 2) * 64
                        tbs = slice(tb * 512, (tb + 1) * 512)
                        if tb == 0:
                            kbd_tiles = [None] * NSI
                        # doubled-Q moving tile for this (h, tb)
                        qd = p2.tile([128, 512], F32R, name="qd", tag="qd", bufs=3)
                        nc.sync.dma_start(qd[0:64, :], qT[ti][ro:ro + 64, tbs])
                        nc.sync.dma_start(qd[64:128, :], qT[ti][ro:ro + 64, tbs])
                        e_list = []
                        pc = p2ps.tile([65, 512], F32, name="ctx_ps",
                                       tag="ctx_ps", bufs=2)
                        for si in range(NSI):
                            if prev is not None:
                                emit_pv(prev, si)
                            if kbd_tiles[si] is None:
                                kb = p2.tile([128, 128], F32R, name="kbd",
                                             tag="kbd", bufs=24)
                                s0 = si * 128
                                nc.sync.dma_start(
                                    kb[0:64, 0:64],
                                    kT[ti][ro:ro + 64, s0:s0 + 64])
                                nc.sync.dma_start(
                                    kb[64:128, 64:128],
                                    kT[ti][ro:ro + 64, s0 + 64:s0 + 128])
                                nc.vector.tensor_copy(kb[0:64, 64:128],
                                                      zeros_t[0:64, :])
                                nc.vector.tensor_copy(kb[64:128, 0:64],
                                                      zeros_t[64:128, :])
                                kbd_tiles[si] = kb
                            ps = p2ps.tile([128, 512], F32, name="sc_ps",
                                           tag="sc_ps", bufs=4)
                            nc.tensor.matmul(ps[:], kbd_tiles[si][:], qd[:],
                                             start=True, stop=True)
                            et = p2.tile([128, 512], F32R, name="e_t",
                                         tag="e_t", bufs=16)
                            nc.scalar.activation(et[:], ps[:], Exp, scale=SCALE)
                            e_list.append(et)
                        if prev is not None:
                            emit_norm(prev)
                        prev = (h, tb, e_list, pc)
                    for si in range(NSI):
                        emit_pv(prev, si)
                    emit_norm(prev)

                # ---------------- Phase 3: out-projection ----------------
                with tc.tile_pool(name="p3", bufs=1) as p3, \
                     tc.tile_pool(name="p3ps", bufs=1, space="PSUM") as p3ps:
                    wo_sb = [p3.tile([128, D], F32R, name=f"wo{c}", tag=f"wo{c}")
                             for c in range(NDT)]
                    for c in range(NDT):
                        nc.sync.dma_start(wo_sb[c][:],
                                          wo[c * 128:(c + 1) * 128, :].bitcast(F32R))
                    bo_sb = p3.tile([128, D], F32, name="bo_sb", tag="bo_sb")
                    nc.sync.dma_start(bo_sb[:], bo[:])
                    for tt in range(NTT):
                        py = p3ps.tile([128, D], F32, name="y_ps", tag="y_ps", bufs=2)
                        for ci in range(NDT):
                            for nh in range(2):
                                nc.tensor.matmul(
                                    py[:, nh * 512:(nh + 1) * 512],
                                    ctx[ci][:, tt * 128:(tt + 1) * 128],
                                    wo_sb[ci][:, nh * 512:(nh + 1) * 512],
                                    start=(ci == 0), stop=(ci == NDT - 1))
                        yt = p3.tile([128, D], F32, name="y_t", tag="y_t", bufs=3)
                        nc.vector.tensor_add(yt[:], py[:], bo_sb[:])
                        nc.sync.dma_start(y[tt * 128:(tt + 1) * 128, :], yt[:])

    nc.compile()
    return nc


_PROGRAM = None


def _get_program():
    global _PROGRAM
    if _PROGRAM is None:
        _PROGRAM = build_program()
    return _PROGRAM


def make_in_maps(x, w_qkv, b_qkv, w_out, b_out):
    x = np.ascontiguousarray(np.asarray(x, dtype=np.float32))
    w_qkv = np.asarray(w_qkv, dtype=np.float32)
    b_qkv = np.asarray(b_qkv, dtype=np.float32)
    w_out = np.asarray(w_out, dtype=np.float32)
    b_out = np.asarray(b_out, dtype=np.float32)

    in_maps = []
    for core in range(NCORES):
        b, g = divmod(core, G)
        gs = slice(g * HD, (g + 1) * HD)
        bo_part = b_out if g == 0 else np.zeros_like(b_out)
        in_maps.append({
            "xt": np.ascontiguousarray(x[b].T),
            "wq": np.ascontiguousarray(w_qkv[:, 0 * D:1 * D][:, gs]),
            "wk": np.ascontiguousarray(w_qkv[:, 1 * D:2 * D][:, gs]),
            "wv": np.ascontiguousarray(w_qkv[:, 2 * D:3 * D][:, gs]),
            "bq": np.ascontiguousarray(b_qkv[0 * D:1 * D][gs].reshape(HD, 1)),
            "bk": np.ascontiguousarray(b_qkv[1 * D:2 * D][gs].reshape(HD, 1)),
            "bv": np.ascontiguousarray(
                np.broadcast_to(b_qkv[2 * D:3 * D][gs], (128, HD))),
            "wo": np.ascontiguousarray(w_out[gs, :]),
            "bo": np.ascontiguousarray(np.broadcast_to(bo_part, (128, D))),
        })
    return in_maps


def run(inputs, trace=False, tmpdir=None):
    nc = _get_program()
    in_maps = make_in_maps(**inputs)
    res = run_bass_kernel_spmd(nc, in_maps, list(range(NCORES)),
                               trace=trace, tmpdir=tmpdir)
    parts = [np.asarray(res.results[c]["y"]) for c in range(NCORES)]
    out = np.empty((B, T, D), dtype=np.float32)
    for b in range(B):
        out[b] = parts[b * G + 0] + parts[b * G + 1]
    return out, res


def kernel(**inputs) -> np.ndarray:
    out, _ = run(inputs, trace=False)
    return out
